# revision 1
# baseline (speedup 1.0000x reference)
"""VQ codebook (HardSOM) forward on 8 TRN2 NeuronCores.

Data-parallel over flattened tokens N=B*T=65536 -> 8 shards of 8192.
Codebook w [2048, 512] replicated per core.

Per core, per 128-token tile:
  scores[n,k] = 2*x_n.w_k - ||w_k||^2  (argmax == argmin of squared distance)
  computed as 3 bf16 matmuls (hi*hi + hi*lo + lo*hi split) + bf16-split bias row,
  argmax via DVE max8/max_index, quantized rows gathered by indirect DMA,
  loss partials via ||x||^2 - max_score (ACT square-accumulate).
Host: final scalar reductions (loss, perplexity) + shard assembly.
"""
import numpy as np
import ml_dtypes

B, T, D, K = 64, 1024, 512, 2048
N = B * T
NCORES = 8
SHARD = N // NCORES          # 8192 tokens per core
P = 128                      # partition dim / tokens per tile
NTILES = SHARD // P          # 64
DCH = D // P                 # 4 d-chunks
CCH = 4                      # code chunks of 512
CW = K // CCH                # 512 codes per chunk
COMMITMENT_COST = 0.25

_CACHE = {}


def _patch_multiwait_split():
    """This walrus build rejects instructions carrying >1 sem waits
    ("Too many sync wait commands" on Tile's final Drain). Split extra waits
    into standalone single-wait EventSemaphore instructions ahead of the
    owning instruction, at the BIR-JSON level just before walrus."""
    import concourse.bass2jax as bass2jax
    if getattr(bass2jax, "_mw_split_installed", False):
        return
    import orjson
    orig = bass2jax.compile_bir_kernel

    def _split(bir_json: bytes) -> bytes:
        d = orjson.loads(bir_json)
        ctr = [0]
        for fn in d.get("functions", []):
            for bb in fn.get("blocks", []):
                insts = bb.get("instructions", [])
                out = []
                for ins in insts:
                    si = ins.get("sync_info")
                    waits = (si or {}).get("on_wait") or []
                    if len(waits) > 1:
                        for w in waits:
                            ctr[0] += 1
                            out.append({
                                "name": f"{ins['name']}-mw{ctr[0]}",
                                "opcode": "EventSemaphore",
                                "engine": ins.get("engine", "SP"),
                                "ins": [], "outs": [],
                                "sync_info": {"on_update": [], "on_wait": [w]},
                            })
                        si["on_wait"] = []
                    out.append(ins)
                bb["instructions"] = out
        return orjson.dumps(d)

    def wrapper(bir_json, tmpdir, neff_name="file.neff"):
        return orig(_split(bir_json), tmpdir, neff_name=neff_name)

    bass2jax.compile_bir_kernel = wrapper
    bass2jax._mw_split_installed = True


def _build_nc():
    import concourse.bass as bass
    import concourse.mybir as mybir
    import concourse.tile as tile

    f32 = mybir.dt.float32
    bf16 = mybir.dt.bfloat16
    i32 = mybir.dt.int32
    u32 = mybir.dt.uint32

    nc = bass.Bass()
    x_d = nc.dram_tensor("x", [SHARD, D], f32, kind="ExternalInput")
    w_d = nc.dram_tensor("w", [K, D], f32, kind="ExternalInput")
    w2hi_d = nc.dram_tensor("w2hi", [DCH, P, K], bf16, kind="ExternalInput")
    w2lo_d = nc.dram_tensor("w2lo", [DCH, P, K], bf16, kind="ExternalInput")
    negs_d = nc.dram_tensor("negs3", [3, K], bf16, kind="ExternalInput")
    ones_d = nc.dram_tensor("ones3", [3, P], bf16, kind="ExternalInput")
    ident_d = nc.dram_tensor("ident", [P, P], f32, kind="ExternalInput")
    q_d = nc.dram_tensor("q", [SHARD, D], f32, kind="ExternalOutput")
    idx_d = nc.dram_tensor("idx", [SHARD, 1], i32, kind="ExternalOutput")
    stats_d = nc.dram_tensor("stats", [P, 2], f32, kind="ExternalOutput")

    with tile.TileContext(nc) as tc:
        with (
            tc.tile_pool(name="const", bufs=1) as cp,
            tc.tile_pool(name="work", bufs=3) as wp,
            tc.tile_pool(name="scores", bufs=2) as sp,
            tc.tile_pool(name="psum", bufs=2, space="PSUM") as pp,
        ):
            w2hi = cp.tile([P, DCH * K], bf16)
            w2lo = cp.tile([P, DCH * K], bf16)
            for d in range(DCH):
                nc.sync.dma_start(w2hi[:, d * K:(d + 1) * K], w2hi_d[d])
                nc.sync.dma_start(w2lo[:, d * K:(d + 1) * K], w2lo_d[d])
            negs = cp.tile([3, K], bf16)
            nc.sync.dma_start(negs[:], negs_d[:])
            ones3 = cp.tile([3, P], bf16)
            nc.sync.dma_start(ones3[:], ones_d[:])
            ident = cp.tile([P, P], f32)
            nc.sync.dma_start(ident[:], ident_d[:])

            xsq_acc = cp.tile([P, 1], f32)
            sc_acc = cp.tile([P, 1], f32)
            nc.vector.memset(xsq_acc[:], 0.0)
            nc.vector.memset(sc_acc[:], 0.0)

            for i in range(NTILES):
                n0 = i * P
                x_nat = wp.tile([P, D], f32)
                nc.sync.dma_start(x_nat[:], x_d[n0:n0 + P, :])

                pc = pp.tile([P, K], f32)
                # transpose x tile chunkwise into psum bank 0 region
                for d in range(DCH):
                    nc.tensor.transpose(
                        pc[:, d * P:(d + 1) * P],
                        in_=x_nat[:, d * P:(d + 1) * P],
                        identity=ident[:],
                    )
                # split xT into bf16 hi/lo
                xhi = wp.tile([P, D], bf16)
                xlo = wp.tile([P, D], bf16)
                nc.vector.tensor_copy(xhi[:], pc[:, 0:D])
                nc.vector.tensor_tensor(
                    out=xlo[:], in0=pc[:, 0:D], in1=xhi[:],
                    op=mybir.AluOpType.subtract,
                )
                # sum of squares of x (for loss), on ACT
                sq_scr = wp.tile([P, D], f32)
                xsq_p = wp.tile([P, 1], f32)
                nc.scalar.activation(
                    out=sq_scr[:], in_=x_nat[:],
                    func=mybir.ActivationFunctionType.Square,
                    accum_out=xsq_p[:],
                )
                nc.vector.tensor_add(xsq_acc[:], xsq_acc[:], xsq_p[:])

                # scores: bias + split3 matmuls, 4 code chunks
                for c in range(CCH):
                    seg = pc[:, c * CW:(c + 1) * CW]
                    nc.tensor.matmul(
                        seg, lhsT=ones3[:], rhs=negs[:, c * CW:(c + 1) * CW],
                        start=True, stop=False,
                    )
                    for d in range(DCH):
                        lh = xhi[:, d * P:(d + 1) * P]
                        ll = xlo[:, d * P:(d + 1) * P]
                        rh = w2hi[:, d * K + c * CW: d * K + (c + 1) * CW]
                        rl = w2lo[:, d * K + c * CW: d * K + (c + 1) * CW]
                        last = (d == DCH - 1)
                        nc.tensor.matmul(seg, lhsT=lh, rhs=rh, start=False, stop=False)
                        nc.tensor.matmul(seg, lhsT=lh, rhs=rl, start=False, stop=False)
                        nc.tensor.matmul(seg, lhsT=ll, rhs=rh, start=False, stop=last)

                scores = sp.tile([P, K], f32)
                nc.scalar.copy(scores[:], pc[:, 0:K])
                mx = wp.tile([P, 8], f32)
                mi = wp.tile([P, 8], u32)
                nc.vector.max(out=mx[:], in_=scores[:])
                nc.vector.max_index(out=mi[:], in_max=mx[:], in_values=scores[:])
                nc.vector.tensor_add(sc_acc[:], sc_acc[:], mx[:, 0:1])

                idx32 = wp.tile([P, 1], i32)
                nc.vector.tensor_copy(idx32[:], mi[:, 0:1])
                nc.sync.dma_start(idx_d[n0:n0 + P, :], idx32[:])

                q_t = wp.tile([P, D], f32)
                nc.gpsimd.indirect_dma_start(
                    out=q_t[:], out_offset=None,
                    in_=w_d[:],
                    in_offset=bass.IndirectOffsetOnAxis(ap=idx32[:, 0:1], axis=0),
                )
                nc.sync.dma_start(q_d[n0:n0 + P, :], q_t[:])

            nc.sync.dma_start(stats_d[:, 0:1], xsq_acc[:])
            nc.sync.dma_start(stats_d[:, 1:2], sc_acc[:])
    return nc


def _get_nc():
    if "nc" not in _CACHE:
        _patch_multiwait_split()
        _CACHE["nc"] = _build_nc()
    return _CACHE["nc"]


def kernel(inputs, w):
    from concourse.bass_utils import run_bass_kernel_spmd

    inputs = np.ascontiguousarray(np.asarray(inputs, dtype=np.float32))
    w = np.ascontiguousarray(np.asarray(w, dtype=np.float32))
    nc = _get_nc()

    # host-side replicated codebook constants (O(K*D), ~1/128 of kernel flops)
    w2 = (2.0 * w.T).astype(np.float32)                    # [D, K]
    w2hi = w2.astype(ml_dtypes.bfloat16)
    w2lo = (w2 - w2hi.astype(np.float32)).astype(ml_dtypes.bfloat16)
    w2hi = np.ascontiguousarray(w2hi.reshape(DCH, P, K))
    w2lo = np.ascontiguousarray(w2lo.reshape(DCH, P, K))
    s = (w.astype(np.float64) ** 2).sum(axis=1)            # ||w_k||^2
    b = -s
    b1 = b.astype(ml_dtypes.bfloat16)
    b2 = (b - b1.astype(np.float64)).astype(ml_dtypes.bfloat16)
    b3 = (b - b1.astype(np.float64) - b2.astype(np.float64)).astype(ml_dtypes.bfloat16)
    negs3 = np.ascontiguousarray(np.stack([b1, b2, b3]))   # [3, K] bf16
    ones3 = np.ones((3, P), dtype=ml_dtypes.bfloat16)
    ident = np.eye(P, dtype=np.float32)

    flat = inputs.reshape(N, D)
    in_maps = []
    for ci in range(NCORES):
        in_maps.append({
            "x": flat[ci * SHARD:(ci + 1) * SHARD],
            "w": w, "w2hi": w2hi, "w2lo": w2lo,
            "negs3": negs3, "ones3": ones3, "ident": ident,
        })

    res = run_bass_kernel_spmd(nc, in_maps, core_ids=list(range(NCORES))).results

    q = np.concatenate([res[ci]["q"] for ci in range(NCORES)], axis=0)
    idx = np.concatenate([res[ci]["idx"] for ci in range(NCORES)], axis=0)
    xsq_tot = float(sum(res[ci]["stats"][:, 0].astype(np.float64).sum() for ci in range(NCORES)))
    sc_tot = float(sum(res[ci]["stats"][:, 1].astype(np.float64).sum() for ci in range(NCORES)))

    loss = np.float32(COMMITMENT_COST * (xsq_tot - sc_tot) / (N * D))
    counts = np.bincount(idx.ravel().astype(np.int64), minlength=K).astype(np.float64)
    p = counts / N
    perplexity = np.float32(np.exp(-(p * np.log(p + 1e-10)).sum()))

    quantized_st = q.reshape(B, T, D)
    return loss, quantized_st, perplexity, idx.astype(np.int32)


# revision 2
# speedup vs baseline: 1.4805x; 1.4805x over previous
"""VQ codebook (HardSOM) forward on 8 TRN2 NeuronCores.

Data-parallel over flattened tokens N=B*T=65536 -> 8 shards of 8192.
Codebook w [2048, 512] replicated per core.

Per core, per 128-token tile:
  scores[n,k] = 2*x_n.w_k - ||w_k||^2  (argmax == argmin of squared distance)
  computed as 3 bf16 matmuls (hi*hi + hi*lo + lo*hi split) + bf16-split bias row,
  argmax via DVE max8/max_index, quantized rows gathered by indirect DMA,
  loss partials via ||x||^2 - max_score (ACT square-accumulate).
Host: final scalar reductions (loss, perplexity) + shard assembly.
"""
import numpy as np
import ml_dtypes

B, T, D, K = 64, 1024, 512, 2048
N = B * T
NCORES = 8
SHARD = N // NCORES          # 8192 tokens per core
P = 128                      # partition dim / tokens per tile
NTILES = SHARD // P          # 64
DCH = D // P                 # 4 d-chunks
CCH = 4                      # code chunks of 512
CW = K // CCH                # 512 codes per chunk
COMMITMENT_COST = 0.25

_CACHE = {}


def _patch_multiwait_split():
    """This walrus build rejects instructions carrying >1 sem waits
    ("Too many sync wait commands" on Tile's final Drain). Split extra waits
    into standalone single-wait EventSemaphore instructions ahead of the
    owning instruction, at the BIR-JSON level just before walrus."""
    import concourse.bass2jax as bass2jax
    if getattr(bass2jax, "_mw_split_installed", False):
        return
    import orjson
    orig = bass2jax.compile_bir_kernel

    def _split(bir_json: bytes) -> bytes:
        d = orjson.loads(bir_json)
        ctr = [0]
        for fn in d.get("functions", []):
            for bb in fn.get("blocks", []):
                insts = bb.get("instructions", [])
                out = []
                for ins in insts:
                    si = ins.get("sync_info")
                    waits = (si or {}).get("on_wait") or []
                    if len(waits) > 1:
                        for w in waits:
                            ctr[0] += 1
                            out.append({
                                "name": f"{ins['name']}-mw{ctr[0]}",
                                "opcode": "EventSemaphore",
                                "engine": ins.get("engine", "SP"),
                                "ins": [], "outs": [],
                                "sync_info": {"on_update": [], "on_wait": [w]},
                            })
                        si["on_wait"] = []
                    out.append(ins)
                bb["instructions"] = out
        return orjson.dumps(d)

    def wrapper(bir_json, tmpdir, neff_name="file.neff"):
        return orig(_split(bir_json), tmpdir, neff_name=neff_name)

    bass2jax.compile_bir_kernel = wrapper
    bass2jax._mw_split_installed = True


def _build_nc():
    import concourse.bass as bass
    import concourse.mybir as mybir
    import concourse.tile as tile

    f32 = mybir.dt.float32
    bf16 = mybir.dt.bfloat16
    i32 = mybir.dt.int32
    u32 = mybir.dt.uint32

    nc = bass.Bass()
    x_d = nc.dram_tensor("x", [SHARD, D], f32, kind="ExternalInput")
    w_d = nc.dram_tensor("w", [K, D], f32, kind="ExternalInput")
    w2hi_d = nc.dram_tensor("w2hi", [DCH, P, K], bf16, kind="ExternalInput")
    w2lo_d = nc.dram_tensor("w2lo", [DCH, P, K], bf16, kind="ExternalInput")
    negs_d = nc.dram_tensor("negs3", [3, K], bf16, kind="ExternalInput")
    ones_d = nc.dram_tensor("ones3", [3, P], bf16, kind="ExternalInput")
    ident_d = nc.dram_tensor("ident", [P, P], f32, kind="ExternalInput")
    q_d = nc.dram_tensor("q", [SHARD, D], f32, kind="ExternalOutput")
    idx_d = nc.dram_tensor("idx", [SHARD, 1], i32, kind="ExternalOutput")
    stats_d = nc.dram_tensor("stats", [P, 2], f32, kind="ExternalOutput")

    with tile.TileContext(nc) as tc:
        with (
            tc.tile_pool(name="const", bufs=1) as cp,
            tc.tile_pool(name="work", bufs=3) as wp,
            tc.tile_pool(name="scores", bufs=2) as sp,
            tc.tile_pool(name="psum", bufs=2, space="PSUM") as pp,
        ):
            w2hi = cp.tile([P, DCH * K], bf16)
            w2lo = cp.tile([P, DCH * K], bf16)
            for d in range(DCH):
                nc.sync.dma_start(w2hi[:, d * K:(d + 1) * K], w2hi_d[d])
                nc.sync.dma_start(w2lo[:, d * K:(d + 1) * K], w2lo_d[d])
            negs = cp.tile([3, K], bf16)
            nc.sync.dma_start(negs[:], negs_d[:])
            ones3 = cp.tile([3, P], bf16)
            nc.sync.dma_start(ones3[:], ones_d[:])
            ident = cp.tile([P, P], f32)
            nc.sync.dma_start(ident[:], ident_d[:])

            xsq_acc = cp.tile([P, 1], f32)
            sc_acc = cp.tile([P, 1], f32)
            nc.vector.memset(xsq_acc[:], 0.0)
            nc.vector.memset(sc_acc[:], 0.0)

            for i in range(NTILES):
                n0 = i * P
                x_nat = wp.tile([P, D], f32)
                nc.sync.dma_start(x_nat[:], x_d[n0:n0 + P, :])

                pc = pp.tile([P, K], f32)
                # transpose x tile chunkwise into psum bank 0 region
                for d in range(DCH):
                    nc.tensor.transpose(
                        pc[:, d * P:(d + 1) * P],
                        in_=x_nat[:, d * P:(d + 1) * P],
                        identity=ident[:],
                    )
                # split xT into bf16 hi/lo
                xhi = wp.tile([P, D], bf16)
                xlo = wp.tile([P, D], bf16)
                nc.vector.tensor_copy(xhi[:], pc[:, 0:D])
                nc.vector.tensor_tensor(
                    out=xlo[:], in0=pc[:, 0:D], in1=xhi[:],
                    op=mybir.AluOpType.subtract,
                )
                # sum of squares of x (for loss), on ACT
                sq_scr = wp.tile([P, D], f32)
                xsq_p = wp.tile([P, 1], f32)
                nc.scalar.activation(
                    out=sq_scr[:], in_=x_nat[:],
                    func=mybir.ActivationFunctionType.Square,
                    accum_out=xsq_p[:],
                )
                nc.vector.tensor_add(xsq_acc[:], xsq_acc[:], xsq_p[:])

                # scores: bias + split3 matmuls, weight-stationary order
                # (each lhsT reused across all 4 code chunks back-to-back).
                # bias c=0 last: bank0 was the transpose scratch, so PE can run
                # bias c=1..3 while DVE finishes the hi/lo split.
                for c in (1, 2, 3, 0):
                    nc.tensor.matmul(
                        pc[:, c * CW:(c + 1) * CW],
                        lhsT=ones3[:], rhs=negs[:, c * CW:(c + 1) * CW],
                        start=True, stop=False, skip_group_check=True,
                    )
                for d in range(DCH):
                    lh = xhi[:, d * P:(d + 1) * P]
                    for c in range(CCH):
                        rh = w2hi[:, d * K + c * CW: d * K + (c + 1) * CW]
                        nc.tensor.matmul(
                            pc[:, c * CW:(c + 1) * CW], lhsT=lh, rhs=rh,
                            start=False, stop=False, skip_group_check=True,
                        )
                    for c in range(CCH):
                        rl = w2lo[:, d * K + c * CW: d * K + (c + 1) * CW]
                        nc.tensor.matmul(
                            pc[:, c * CW:(c + 1) * CW], lhsT=lh, rhs=rl,
                            start=False, stop=False, skip_group_check=True,
                        )
                for d in range(DCH):
                    ll = xlo[:, d * P:(d + 1) * P]
                    last = (d == DCH - 1)
                    for c in range(CCH):
                        rh = w2hi[:, d * K + c * CW: d * K + (c + 1) * CW]
                        nc.tensor.matmul(
                            pc[:, c * CW:(c + 1) * CW], lhsT=ll, rhs=rh,
                            start=False, stop=last, skip_group_check=True,
                        )

                scores = sp.tile([P, K], f32)
                nc.scalar.copy(scores[:], pc[:, 0:K])
                mx = wp.tile([P, 8], f32)
                mi = wp.tile([P, 8], u32)
                nc.vector.max(out=mx[:], in_=scores[:])
                nc.vector.max_index(out=mi[:], in_max=mx[:], in_values=scores[:])
                nc.vector.tensor_add(sc_acc[:], sc_acc[:], mx[:, 0:1])

                idx32 = wp.tile([P, 1], i32)
                nc.vector.tensor_copy(idx32[:], mi[:, 0:1])
                nc.sync.dma_start(idx_d[n0:n0 + P, :], idx32[:])

                q_t = wp.tile([P, D], f32)
                nc.gpsimd.indirect_dma_start(
                    out=q_t[:], out_offset=None,
                    in_=w_d[:],
                    in_offset=bass.IndirectOffsetOnAxis(ap=idx32[:, 0:1], axis=0),
                )
                nc.sync.dma_start(q_d[n0:n0 + P, :], q_t[:])

            nc.sync.dma_start(stats_d[:, 0:1], xsq_acc[:])
            nc.sync.dma_start(stats_d[:, 1:2], sc_acc[:])
    return nc


def _get_nc():
    if "nc" not in _CACHE:
        _patch_multiwait_split()
        _CACHE["nc"] = _build_nc()
    return _CACHE["nc"]


def kernel(inputs, w):
    from concourse.bass_utils import run_bass_kernel_spmd

    inputs = np.ascontiguousarray(np.asarray(inputs, dtype=np.float32))
    w = np.ascontiguousarray(np.asarray(w, dtype=np.float32))
    nc = _get_nc()

    # host-side replicated codebook constants (O(K*D), ~1/128 of kernel flops)
    w2 = (2.0 * w.T).astype(np.float32)                    # [D, K]
    w2hi = w2.astype(ml_dtypes.bfloat16)
    w2lo = (w2 - w2hi.astype(np.float32)).astype(ml_dtypes.bfloat16)
    w2hi = np.ascontiguousarray(w2hi.reshape(DCH, P, K))
    w2lo = np.ascontiguousarray(w2lo.reshape(DCH, P, K))
    s = (w.astype(np.float64) ** 2).sum(axis=1)            # ||w_k||^2
    b = -s
    b1 = b.astype(ml_dtypes.bfloat16)
    b2 = (b - b1.astype(np.float64)).astype(ml_dtypes.bfloat16)
    b3 = (b - b1.astype(np.float64) - b2.astype(np.float64)).astype(ml_dtypes.bfloat16)
    negs3 = np.ascontiguousarray(np.stack([b1, b2, b3]))   # [3, K] bf16
    ones3 = np.ones((3, P), dtype=ml_dtypes.bfloat16)
    ident = np.eye(P, dtype=np.float32)

    flat = inputs.reshape(N, D)
    in_maps = []
    for ci in range(NCORES):
        in_maps.append({
            "x": flat[ci * SHARD:(ci + 1) * SHARD],
            "w": w, "w2hi": w2hi, "w2lo": w2lo,
            "negs3": negs3, "ones3": ones3, "ident": ident,
        })

    res = run_bass_kernel_spmd(nc, in_maps, core_ids=list(range(NCORES))).results

    q = np.concatenate([res[ci]["q"] for ci in range(NCORES)], axis=0)
    idx = np.concatenate([res[ci]["idx"] for ci in range(NCORES)], axis=0)
    xsq_tot = float(sum(res[ci]["stats"][:, 0].astype(np.float64).sum() for ci in range(NCORES)))
    sc_tot = float(sum(res[ci]["stats"][:, 1].astype(np.float64).sum() for ci in range(NCORES)))

    loss = np.float32(COMMITMENT_COST * (xsq_tot - sc_tot) / (N * D))
    counts = np.bincount(idx.ravel().astype(np.int64), minlength=K).astype(np.float64)
    p = counts / N
    perplexity = np.float32(np.exp(-(p * np.log(p + 1e-10)).sum()))

    quantized_st = q.reshape(B, T, D)
    return loss, quantized_st, perplexity, idx.astype(np.int32)


# revision 3
# speedup vs baseline: 1.4971x; 1.0112x over previous
"""VQ codebook (HardSOM) forward on 8 TRN2 NeuronCores.

Data-parallel over flattened tokens N=B*T=65536 -> 8 shards of 8192.
Codebook w [2048, 512] replicated per core.

Per core, per 128-token tile:
  scores[n,k] = 2*x_n.w_k - ||w_k||^2  (argmax == argmin of squared distance)
  computed as 3 bf16 matmuls (hi*hi + hi*lo + lo*hi split) + bf16-split bias row,
  argmax via DVE max8/max_index, quantized rows gathered by indirect DMA,
  loss partials via ||x||^2 - max_score (ACT square-accumulate).
Host: final scalar reductions (loss, perplexity) + shard assembly.
"""
import numpy as np
import ml_dtypes

B, T, D, K = 64, 1024, 512, 2048
N = B * T
NCORES = 8
SHARD = N // NCORES          # 8192 tokens per core
P = 128                      # partition dim / tokens per tile
NTILES = SHARD // P          # 64
DCH = D // P                 # 4 d-chunks
CCH = 4                      # code chunks of 512
CW = K // CCH                # 512 codes per chunk
COMMITMENT_COST = 0.25

_CACHE = {}


def _patch_multiwait_split():
    """This walrus build rejects instructions carrying >1 sem waits
    ("Too many sync wait commands" on Tile's final Drain). Split extra waits
    into standalone single-wait EventSemaphore instructions ahead of the
    owning instruction, at the BIR-JSON level just before walrus."""
    import concourse.bass2jax as bass2jax
    if getattr(bass2jax, "_mw_split_installed", False):
        return
    import orjson
    orig = bass2jax.compile_bir_kernel

    def _split(bir_json: bytes) -> bytes:
        d = orjson.loads(bir_json)
        ctr = [0]
        for fn in d.get("functions", []):
            for bb in fn.get("blocks", []):
                insts = bb.get("instructions", [])
                out = []
                for ins in insts:
                    si = ins.get("sync_info")
                    waits = (si or {}).get("on_wait") or []
                    if len(waits) > 1:
                        for w in waits:
                            ctr[0] += 1
                            out.append({
                                "name": f"{ins['name']}-mw{ctr[0]}",
                                "opcode": "EventSemaphore",
                                "engine": ins.get("engine", "SP"),
                                "ins": [], "outs": [],
                                "sync_info": {"on_update": [], "on_wait": [w]},
                            })
                        si["on_wait"] = []
                    out.append(ins)
                bb["instructions"] = out
        return orjson.dumps(d)

    def wrapper(bir_json, tmpdir, neff_name="file.neff"):
        return orig(_split(bir_json), tmpdir, neff_name=neff_name)

    bass2jax.compile_bir_kernel = wrapper
    bass2jax._mw_split_installed = True


def _build_nc():
    import concourse.bass as bass
    import concourse.mybir as mybir
    import concourse.tile as tile

    f32 = mybir.dt.float32
    bf16 = mybir.dt.bfloat16
    i32 = mybir.dt.int32
    u32 = mybir.dt.uint32

    nc = bass.Bass()
    x_d = nc.dram_tensor("x", [SHARD, D], f32, kind="ExternalInput")
    w_d = nc.dram_tensor("w", [K, D], f32, kind="ExternalInput")
    w2hi_d = nc.dram_tensor("w2hi", [DCH, P, K], bf16, kind="ExternalInput")
    w2lo_d = nc.dram_tensor("w2lo", [DCH, P, K], bf16, kind="ExternalInput")
    negs_d = nc.dram_tensor("negs3", [3, K], bf16, kind="ExternalInput")
    ones_d = nc.dram_tensor("ones3", [3, P], bf16, kind="ExternalInput")
    ident_d = nc.dram_tensor("ident", [P, P], f32, kind="ExternalInput")
    q_d = nc.dram_tensor("q", [SHARD, D], f32, kind="ExternalOutput")
    idx_d = nc.dram_tensor("idx", [SHARD, 1], i32, kind="ExternalOutput")
    stats_d = nc.dram_tensor("stats", [P, 2], f32, kind="ExternalOutput")

    with tile.TileContext(nc) as tc:
        with (
            tc.tile_pool(name="const", bufs=1) as cp,
            tc.tile_pool(name="work", bufs=4) as wp,
            tc.tile_pool(name="scores", bufs=3) as sp,
            tc.tile_pool(name="psum", bufs=2, space="PSUM") as pp,
        ):
            w2hi = cp.tile([P, DCH * K], bf16)
            w2lo = cp.tile([P, DCH * K], bf16)
            for d in range(DCH):
                nc.sync.dma_start(w2hi[:, d * K:(d + 1) * K], w2hi_d[d])
                nc.sync.dma_start(w2lo[:, d * K:(d + 1) * K], w2lo_d[d])
            negs = cp.tile([3, K], bf16)
            nc.sync.dma_start(negs[:], negs_d[:])
            ones3 = cp.tile([3, P], bf16)
            nc.sync.dma_start(ones3[:], ones_d[:])
            ident = cp.tile([P, P], f32)
            nc.sync.dma_start(ident[:], ident_d[:])

            xsq_acc = cp.tile([P, 1], f32)
            sc_acc = cp.tile([P, 1], f32)
            nc.vector.memset(xsq_acc[:], 0.0)
            nc.vector.memset(sc_acc[:], 0.0)

            for i in range(NTILES):
                n0 = i * P
                x_nat = wp.tile([P, D], f32)
                nc.sync.dma_start(x_nat[:], x_d[n0:n0 + P, :])

                pc = pp.tile([P, K], f32)
                # transpose x tile chunkwise into psum bank 0 region
                for d in range(DCH):
                    nc.tensor.transpose(
                        pc[:, d * P:(d + 1) * P],
                        in_=x_nat[:, d * P:(d + 1) * P],
                        identity=ident[:],
                    )
                # split xT into bf16 hi/lo
                xhi = wp.tile([P, D], bf16)
                xlo = wp.tile([P, D], bf16)
                nc.vector.tensor_copy(xhi[:], pc[:, 0:D])
                nc.vector.tensor_tensor(
                    out=xlo[:], in0=pc[:, 0:D], in1=xhi[:],
                    op=mybir.AluOpType.subtract,
                )
                # sum of squares of x (for loss), on ACT
                sq_scr = wp.tile([P, D], f32)
                xsq_p = wp.tile([P, 1], f32)
                nc.scalar.activation(
                    out=sq_scr[:], in_=x_nat[:],
                    func=mybir.ActivationFunctionType.Square,
                    accum_out=xsq_p[:],
                )
                nc.vector.tensor_add(xsq_acc[:], xsq_acc[:], xsq_p[:])

                # scores: bias + split3 matmuls, weight-stationary order
                # (each lhsT reused across all 4 code chunks back-to-back).
                # bias c=0 last: bank0 was the transpose scratch, so PE can run
                # bias c=1..3 while DVE finishes the hi/lo split.
                for c in (1, 2, 3, 0):
                    nc.tensor.matmul(
                        pc[:, c * CW:(c + 1) * CW],
                        lhsT=ones3[:], rhs=negs[:, c * CW:(c + 1) * CW],
                        start=True, stop=False, skip_group_check=True,
                    )
                for d in range(DCH):
                    lh = xhi[:, d * P:(d + 1) * P]
                    for c in range(CCH):
                        rh = w2hi[:, d * K + c * CW: d * K + (c + 1) * CW]
                        nc.tensor.matmul(
                            pc[:, c * CW:(c + 1) * CW], lhsT=lh, rhs=rh,
                            start=False, stop=False, skip_group_check=True,
                        )
                    for c in range(CCH):
                        rl = w2lo[:, d * K + c * CW: d * K + (c + 1) * CW]
                        nc.tensor.matmul(
                            pc[:, c * CW:(c + 1) * CW], lhsT=lh, rhs=rl,
                            start=False, stop=False, skip_group_check=True,
                        )
                for d in range(DCH):
                    ll = xlo[:, d * P:(d + 1) * P]
                    last = (d == DCH - 1)
                    for c in range(CCH):
                        rh = w2hi[:, d * K + c * CW: d * K + (c + 1) * CW]
                        nc.tensor.matmul(
                            pc[:, c * CW:(c + 1) * CW], lhsT=ll, rhs=rh,
                            start=False, stop=last, skip_group_check=True,
                        )

                scores = sp.tile([P, K], f32)
                nc.scalar.copy(scores[:], pc[:, 0:K])
                mx = wp.tile([P, 8], f32)
                mi = wp.tile([P, 8], u32)
                nc.vector.max(out=mx[:], in_=scores[:])
                nc.vector.max_index(out=mi[:], in_max=mx[:], in_values=scores[:])
                nc.vector.tensor_add(sc_acc[:], sc_acc[:], mx[:, 0:1])

                idx32 = wp.tile([P, 1], i32)
                nc.vector.tensor_copy(idx32[:], mi[:, 0:1])
                nc.sync.dma_start(idx_d[n0:n0 + P, :], idx32[:])

                q_t = wp.tile([P, D], f32)
                nc.gpsimd.indirect_dma_start(
                    out=q_t[:], out_offset=None,
                    in_=w_d[:],
                    in_offset=bass.IndirectOffsetOnAxis(ap=idx32[:, 0:1], axis=0),
                )
                nc.sync.dma_start(q_d[n0:n0 + P, :], q_t[:])

            nc.sync.dma_start(stats_d[:, 0:1], xsq_acc[:])
            nc.sync.dma_start(stats_d[:, 1:2], sc_acc[:])
    return nc


def _get_nc():
    if "nc" not in _CACHE:
        _patch_multiwait_split()
        _CACHE["nc"] = _build_nc()
    return _CACHE["nc"]


def kernel(inputs, w):
    from concourse.bass_utils import run_bass_kernel_spmd

    inputs = np.ascontiguousarray(np.asarray(inputs, dtype=np.float32))
    w = np.ascontiguousarray(np.asarray(w, dtype=np.float32))
    nc = _get_nc()

    # host-side replicated codebook constants (O(K*D), ~1/128 of kernel flops)
    w2 = (2.0 * w.T).astype(np.float32)                    # [D, K]
    w2hi = w2.astype(ml_dtypes.bfloat16)
    w2lo = (w2 - w2hi.astype(np.float32)).astype(ml_dtypes.bfloat16)
    w2hi = np.ascontiguousarray(w2hi.reshape(DCH, P, K))
    w2lo = np.ascontiguousarray(w2lo.reshape(DCH, P, K))
    s = (w.astype(np.float64) ** 2).sum(axis=1)            # ||w_k||^2
    b = -s
    b1 = b.astype(ml_dtypes.bfloat16)
    b2 = (b - b1.astype(np.float64)).astype(ml_dtypes.bfloat16)
    b3 = (b - b1.astype(np.float64) - b2.astype(np.float64)).astype(ml_dtypes.bfloat16)
    negs3 = np.ascontiguousarray(np.stack([b1, b2, b3]))   # [3, K] bf16
    ones3 = np.ones((3, P), dtype=ml_dtypes.bfloat16)
    ident = np.eye(P, dtype=np.float32)

    flat = inputs.reshape(N, D)
    in_maps = []
    for ci in range(NCORES):
        in_maps.append({
            "x": flat[ci * SHARD:(ci + 1) * SHARD],
            "w": w, "w2hi": w2hi, "w2lo": w2lo,
            "negs3": negs3, "ones3": ones3, "ident": ident,
        })

    res = run_bass_kernel_spmd(nc, in_maps, core_ids=list(range(NCORES))).results

    q = np.concatenate([res[ci]["q"] for ci in range(NCORES)], axis=0)
    idx = np.concatenate([res[ci]["idx"] for ci in range(NCORES)], axis=0)
    xsq_tot = float(sum(res[ci]["stats"][:, 0].astype(np.float64).sum() for ci in range(NCORES)))
    sc_tot = float(sum(res[ci]["stats"][:, 1].astype(np.float64).sum() for ci in range(NCORES)))

    loss = np.float32(COMMITMENT_COST * (xsq_tot - sc_tot) / (N * D))
    counts = np.bincount(idx.ravel().astype(np.int64), minlength=K).astype(np.float64)
    p = counts / N
    perplexity = np.float32(np.exp(-(p * np.log(p + 1e-10)).sum()))

    quantized_st = q.reshape(B, T, D)
    return loss, quantized_st, perplexity, idx.astype(np.int32)


# revision 5
# speedup vs baseline: 1.5216x; 1.0163x over previous
"""VQ codebook (HardSOM) forward on 8 TRN2 NeuronCores.

Data-parallel over flattened tokens N=B*T=65536 -> 8 shards of 8192.
Codebook w [2048, 512] replicated per core.

Per core, per 128-token tile:
  scores[n,k] = 2*x_n.w_k - ||w_k||^2  (argmax == argmin of squared distance)
  computed as 3 bf16 matmuls (hi*hi + hi*lo + lo*hi split) + bf16-split bias row,
  argmax via DVE max8/max_index, quantized rows gathered by indirect DMA,
  loss partials via ||x||^2 - max_score (ACT square-accumulate).
Host: final scalar reductions (loss, perplexity) + shard assembly.
"""
import numpy as np
import ml_dtypes

B, T, D, K = 64, 1024, 512, 2048
N = B * T
NCORES = 8
SHARD = N // NCORES          # 8192 tokens per core
P = 128                      # partition dim / tokens per tile
NTILES = SHARD // P          # 64
DCH = D // P                 # 4 d-chunks
CCH = 4                      # code chunks of 512
CW = K // CCH                # 512 codes per chunk
COMMITMENT_COST = 0.25

_CACHE = {}


def _patch_multiwait_split():
    """This walrus build rejects instructions carrying >1 sem waits
    ("Too many sync wait commands" on Tile's final Drain). Split extra waits
    into standalone single-wait EventSemaphore instructions ahead of the
    owning instruction, at the BIR-JSON level just before walrus."""
    import concourse.bass2jax as bass2jax
    if getattr(bass2jax, "_mw_split_installed", False):
        return
    import orjson
    orig = bass2jax.compile_bir_kernel

    def _split(bir_json: bytes) -> bytes:
        d = orjson.loads(bir_json)
        ctr = [0]
        for fn in d.get("functions", []):
            for bb in fn.get("blocks", []):
                insts = bb.get("instructions", [])
                out = []
                for ins in insts:
                    si = ins.get("sync_info")
                    waits = (si or {}).get("on_wait") or []
                    if len(waits) > 1:
                        for w in waits:
                            ctr[0] += 1
                            out.append({
                                "name": f"{ins['name']}-mw{ctr[0]}",
                                "opcode": "EventSemaphore",
                                "engine": ins.get("engine", "SP"),
                                "ins": [], "outs": [],
                                "sync_info": {"on_update": [], "on_wait": [w]},
                            })
                        si["on_wait"] = []
                    out.append(ins)
                bb["instructions"] = out
        return orjson.dumps(d)

    def wrapper(bir_json, tmpdir, neff_name="file.neff"):
        return orig(_split(bir_json), tmpdir, neff_name=neff_name)

    bass2jax.compile_bir_kernel = wrapper
    bass2jax._mw_split_installed = True


def _build_nc():
    import concourse.bass as bass
    import concourse.mybir as mybir
    import concourse.tile as tile

    f32 = mybir.dt.float32
    bf16 = mybir.dt.bfloat16
    i32 = mybir.dt.int32
    u32 = mybir.dt.uint32

    nc = bass.Bass()
    x_d = nc.dram_tensor("x", [SHARD, D], f32, kind="ExternalInput")
    w_d = nc.dram_tensor("w", [K, D], f32, kind="ExternalInput")
    w2hi_d = nc.dram_tensor("w2hi", [DCH, P, K], bf16, kind="ExternalInput")
    w2lo_d = nc.dram_tensor("w2lo", [DCH, P, K], bf16, kind="ExternalInput")
    negs_d = nc.dram_tensor("negs3", [3, K], bf16, kind="ExternalInput")
    ones_d = nc.dram_tensor("ones3", [3, P], bf16, kind="ExternalInput")
    ident_d = nc.dram_tensor("ident", [P, P], f32, kind="ExternalInput")
    q_d = nc.dram_tensor("q", [SHARD, D], f32, kind="ExternalOutput")
    idx_d = nc.dram_tensor("idx", [SHARD, 1], i32, kind="ExternalOutput")
    stats_d = nc.dram_tensor("stats", [P, 2], f32, kind="ExternalOutput")

    with tile.TileContext(nc) as tc:
        with (
            tc.tile_pool(name="const", bufs=1) as cp,
            tc.tile_pool(name="work", bufs=4) as wp,
            tc.tile_pool(name="scores", bufs=3) as sp,
            tc.tile_pool(name="psum", bufs=2, space="PSUM") as pp,
        ):
            w2hi = cp.tile([P, DCH * K], bf16)
            w2lo = cp.tile([P, DCH * K], bf16)
            for d in range(DCH):
                nc.sync.dma_start(w2hi[:, d * K:(d + 1) * K], w2hi_d[d])
                nc.sync.dma_start(w2lo[:, d * K:(d + 1) * K], w2lo_d[d])
            negs = cp.tile([3, K], bf16)
            nc.sync.dma_start(negs[:], negs_d[:])
            ones3 = cp.tile([3, P], bf16)
            nc.sync.dma_start(ones3[:], ones_d[:])
            ident = cp.tile([P, P], f32)
            nc.sync.dma_start(ident[:], ident_d[:])

            xsq_acc = cp.tile([P, 1], f32)
            sc_acc = cp.tile([P, 1], f32)
            nc.vector.memset(xsq_acc[:], 0.0)
            nc.vector.memset(sc_acc[:], 0.0)

            for i in range(NTILES):
                n0 = i * P
                x_nat = wp.tile([P, D], f32)
                nc.sync.dma_start(x_nat[:], x_d[n0:n0 + P, :])

                pc = pp.tile([P, K], f32)
                # transpose x tile chunkwise into psum bank 0 region
                for d in range(DCH):
                    nc.tensor.transpose(
                        pc[:, d * P:(d + 1) * P],
                        in_=x_nat[:, d * P:(d + 1) * P],
                        identity=ident[:],
                    )
                # split xT into bf16 hi/lo
                xhi = wp.tile([P, D], bf16)
                xlo = wp.tile([P, D], bf16)
                nc.vector.tensor_copy(xhi[:], pc[:, 0:D])
                nc.vector.tensor_tensor(
                    out=xlo[:], in0=pc[:, 0:D], in1=xhi[:],
                    op=mybir.AluOpType.subtract,
                )
                # sum of squares of x (for loss), on ACT
                sq_scr = wp.tile([P, D], f32)
                xsq_p = wp.tile([P, 1], f32)
                nc.scalar.activation(
                    out=sq_scr[:], in_=x_nat[:],
                    func=mybir.ActivationFunctionType.Square,
                    accum_out=xsq_p[:],
                )
                nc.vector.tensor_add(xsq_acc[:], xsq_acc[:], xsq_p[:])

                # scores: bias + split3 matmuls, weight-stationary order
                # (each lhsT reused across all 4 code chunks back-to-back).
                # bias c=0 last: bank0 was the transpose scratch, so PE can run
                # bias c=1..3 while DVE finishes the hi/lo split.
                for c in (1, 2, 3, 0):
                    nc.tensor.matmul(
                        pc[:, c * CW:(c + 1) * CW],
                        lhsT=ones3[:], rhs=negs[:, c * CW:(c + 1) * CW],
                        start=True, stop=False, skip_group_check=True,
                    )
                for d in range(DCH):
                    lh = xhi[:, d * P:(d + 1) * P]
                    for c in range(CCH):
                        rh = w2hi[:, d * K + c * CW: d * K + (c + 1) * CW]
                        nc.tensor.matmul(
                            pc[:, c * CW:(c + 1) * CW], lhsT=lh, rhs=rh,
                            start=False, stop=False, skip_group_check=True,
                        )
                    for c in range(CCH):
                        rl = w2lo[:, d * K + c * CW: d * K + (c + 1) * CW]
                        nc.tensor.matmul(
                            pc[:, c * CW:(c + 1) * CW], lhsT=lh, rhs=rl,
                            start=False, stop=False, skip_group_check=True,
                        )
                for d in range(DCH):
                    ll = xlo[:, d * P:(d + 1) * P]
                    last = (d == DCH - 1)
                    for c in range(CCH):
                        rh = w2hi[:, d * K + c * CW: d * K + (c + 1) * CW]
                        nc.tensor.matmul(
                            pc[:, c * CW:(c + 1) * CW], lhsT=ll, rhs=rh,
                            start=False, stop=last, skip_group_check=True,
                        )

                scores = sp.tile([P, K], f32)
                nc.scalar.copy(scores[:], pc[:, 0:K])
                mx = wp.tile([P, 8], f32)
                mi = wp.tile([P, 8], u32)
                nc.vector.max(out=mx[:], in_=scores[:])
                nc.vector.max_index(out=mi[:], in_max=mx[:], in_values=scores[:])
                nc.vector.tensor_add(sc_acc[:], sc_acc[:], mx[:, 0:1])

                idx32 = wp.tile([P, 1], i32)
                nc.vector.tensor_copy(idx32[:], mi[:, 0:1])
                nc.sync.dma_start(idx_d[n0:n0 + P, :], idx32[:])

                q_t = wp.tile([P, D], f32)
                nc.gpsimd.indirect_dma_start(
                    out=q_t[:], out_offset=None,
                    in_=w_d[:],
                    in_offset=bass.IndirectOffsetOnAxis(ap=idx32[:, 0:1], axis=0),
                )
                nc.sync.dma_start(q_d[n0:n0 + P, :], q_t[:])

            nc.sync.dma_start(stats_d[:, 0:1], xsq_acc[:])
            nc.sync.dma_start(stats_d[:, 1:2], sc_acc[:])
    return nc


def _get_nc():
    if "nc" not in _CACHE:
        _patch_multiwait_split()
        _CACHE["nc"] = _build_nc()
    return _CACHE["nc"]


def kernel(inputs, w):
    from concourse.bass_utils import run_bass_kernel_spmd

    inputs = np.ascontiguousarray(np.asarray(inputs, dtype=np.float32))
    w = np.ascontiguousarray(np.asarray(w, dtype=np.float32))
    nc = _get_nc()

    # host-side replicated codebook constants (O(K*D), ~1/128 of kernel flops)
    w2 = (2.0 * w.T).astype(np.float32)                    # [D, K]
    w2hi = w2.astype(ml_dtypes.bfloat16)
    w2lo = (w2 - w2hi.astype(np.float32)).astype(ml_dtypes.bfloat16)
    w2hi = np.ascontiguousarray(w2hi.reshape(DCH, P, K))
    w2lo = np.ascontiguousarray(w2lo.reshape(DCH, P, K))
    s = (w.astype(np.float64) ** 2).sum(axis=1)            # ||w_k||^2
    b = -s
    b1 = b.astype(ml_dtypes.bfloat16)
    b2 = (b - b1.astype(np.float64)).astype(ml_dtypes.bfloat16)
    b3 = (b - b1.astype(np.float64) - b2.astype(np.float64)).astype(ml_dtypes.bfloat16)
    negs3 = np.ascontiguousarray(np.stack([b1, b2, b3]))   # [3, K] bf16
    ones3 = np.ones((3, P), dtype=ml_dtypes.bfloat16)
    ident = np.eye(P, dtype=np.float32)

    flat = inputs.reshape(N, D)
    in_maps = []
    for ci in range(NCORES):
        in_maps.append({
            "x": flat[ci * SHARD:(ci + 1) * SHARD],
            "w": w, "w2hi": w2hi, "w2lo": w2lo,
            "negs3": negs3, "ones3": ones3, "ident": ident,
        })

    res = run_bass_kernel_spmd(nc, in_maps, core_ids=list(range(NCORES))).results

    q = np.concatenate([res[ci]["q"] for ci in range(NCORES)], axis=0)
    idx = np.concatenate([res[ci]["idx"] for ci in range(NCORES)], axis=0)
    xsq_tot = float(sum(res[ci]["stats"][:, 0].astype(np.float64).sum() for ci in range(NCORES)))
    sc_tot = float(sum(res[ci]["stats"][:, 1].astype(np.float64).sum() for ci in range(NCORES)))

    loss = np.float32(COMMITMENT_COST * (xsq_tot - sc_tot) / (N * D))
    counts = np.bincount(idx.ravel().astype(np.int64), minlength=K).astype(np.float64)
    p = counts / N
    perplexity = np.float32(np.exp(-(p * np.log(p + 1e-10)).sum()))

    quantized_st = q.reshape(B, T, D)
    return loss, quantized_st, perplexity, idx.astype(np.int32)
